# revision 11
# baseline (speedup 1.0000x reference)
"""CatGCN encoder kernel for Trainium2 (8 NeuronCores, SPMD).

  user_h = relu(concat_s A_s @ (item_inputs @ W_s)),  item_h symmetric.

Per core: build the 1-2 bf16 table chunks (V_s = item_inputs@W_s resp.
U_s = user_inputs@W_s) its edge workload gathers from (stage 1, X^T
stationary matmuls over host-pretransposed bf16 inputs), then process
destination groups of 128 rows: indirect-DMA gather of source rows
(bf16, 200B, OOB-skip padding), a one-hot*weight selection matrix P per
128-edge chunk built on DVE (tensor_scalar is_equal*mult vs iota), PE
matmul P^T @ gathered accumulating [128,100] in PSUM, ACT relu, SP
writes the output tile.  Work is split into equal contiguous bands of
(relation, dest-group) atoms -> identical SPMD program on all 8 cores.
Raw Block style; all cross-engine waits are standalone wait_ge (this
toolchain allows only one sync wait per instruction).
"""

import os
import time

import numpy as np
import ml_dtypes
import sys

sys.path.insert(0, "/opt/trn_rl_repo")

# NTFF profiling hook is unavailable in this container; a stray BASS_TRACE
# would crash run_bass_kernel_spmd under axon.
os.environ["BASS_NEVER_TRACE"] = "1"

EXEC_WALL_S = None

import concourse.bass as bass
from concourse import mybir
from concourse.bass_utils import run_bass_kernel_spmd

BF16 = mybir.dt.bfloat16
F32 = mybir.dt.float32
I32 = mybir.dt.int32

NU, NI, D, O, S, E = 100000, 50000, 512, 500, 5, 200000
SUB = O // S          # 100
NCORES = 8
P = 128
KT = D // P           # 4 k-tiles
ROWTILE = 1024
TI = 49               # item row tiles (49*1024 = 50176 >= NI)
TU = 98               # user row tiles (98*1024 = 100352 >= NU)
RI = TI * ROWTILE
RU = TU * ROWTILE
GI = RI // P          # item dest groups per relation (392)
GU = RU // P          # user dest groups per relation (784)
PADIDX = 1 << 28

GB_U = 4              # user groups per gather batch
GB_I = 2              # item groups per gather batch

LAST_RESULTS = None


def _host_prep(user_inputs, item_inputs, weight,
               u_rows, u_cols, u_vals, i_rows, i_cols, i_vals,
               ub_groups, ib_groups, c_u, c_i):
    w16 = np.asarray(weight).astype(ml_dtypes.bfloat16)

    def pretile(x, nt):
        rows = nt * ROWTILE
        xt = np.zeros((D, rows), dtype=ml_dtypes.bfloat16)
        xt[:, : x.shape[0]] = np.asarray(x).astype(ml_dtypes.bfloat16).T
        return np.ascontiguousarray(
            xt.reshape(KT, P, nt, ROWTILE).transpose(2, 1, 0, 3)
        )  # [nt][p][k][c]

    xiT = pretile(item_inputs, TI)
    xuT = pretile(user_inputs, TU)
    iota = np.broadcast_to(np.arange(P, dtype=ml_dtypes.bfloat16), (P, P)).copy()

    u_atoms = [(r, g) for r in range(S) for g in range(GU)]
    i_atoms = [(r, g) for r in range(S) for g in range(GI)]
    while len(u_atoms) < NCORES * ub_groups:
        u_atoms.append((None, None))
    while len(i_atoms) < NCORES * ib_groups:
        i_atoms.append((None, None))

    def bucket(rows_all, cols_all, vals_all, n_groups):
        out = []
        for r in range(S):
            rows = np.asarray(rows_all[r])
            order = np.argsort(rows, kind="stable")
            rs = rows[order]
            cs = np.asarray(cols_all[r])[order]
            vs = np.asarray(vals_all[r])[order]
            g = rs // P
            starts = np.searchsorted(g, np.arange(n_groups))
            ends = np.searchsorted(g, np.arange(n_groups) + 1)
            out.append((rs, cs, vs, starts, ends))
        return out

    ub = bucket(u_rows, u_cols, u_vals, GU)
    ib = bucket(i_rows, i_cols, i_vals, GI)

    in_maps, asm = [], []
    for c in range(NCORES):
        ub_band = u_atoms[c * ub_groups : (c + 1) * ub_groups]
        ib_band = i_atoms[c * ib_groups : (c + 1) * ib_groups]
        urels = sorted({r for r, _ in ub_band if r is not None})
        irels = sorted({r for r, _ in ib_band if r is not None})
        assert len(urels) <= 2 and len(irels) <= 2, (urels, irels)
        while len(urels) < 2:
            urels.append(urels[0] if urels else 0)
        while len(irels) < 2:
            irels.append(irels[0] if irels else 0)
        uslot = {urels[0]: 0, urels[1]: 1}
        islot = {irels[0]: 0, irels[1]: 1}

        def packw(rels):
            cols = np.concatenate(
                [w16[:, r * SUB : (r + 1) * SUB] for r in rels], axis=1
            )  # [512, 200]
            return np.ascontiguousarray(
                cols.reshape(KT, P, 2 * SUB).transpose(1, 0, 2).reshape(P, KT * 2 * SUB)
            )

        def build_meta(band, buckets, slot_of, c_k, gb, table_rows):
            ncols = len(band) * c_k
            idx = np.zeros((P, ncols), dtype=np.int64)
            lr = np.zeros((P, ncols), dtype=np.float32)
            wt = np.zeros((P, ncols), dtype=np.float32)
            for b_pos, (r, g) in enumerate(band):
                if r is None:
                    continue
                rs, cs, vs, starts, ends = buckets[r]
                s0, e0 = int(starts[g]), int(ends[g])
                n = e0 - s0
                assert n <= c_k * P, f"group overflow {n} > {c_k * P}"
                j = np.arange(n)
                pp = j % P
                col = b_pos * c_k + j // P
                idx[pp, col] = slot_of[r] * table_rows + cs[s0:e0]
                lr[pp, col] = (rs[s0:e0] - g * P).astype(np.float32)
                wt[pp, col] = vs[s0:e0]
            return np.ascontiguousarray(
                np.concatenate(
                    [idx.astype(np.int32), lr.view(np.int32), wt.view(np.int32)],
                    axis=1,
                )
            )

        in_maps.append(
            {
                "xiT": xiT, "xuT": xuT,
                "wv": packw(urels), "wu": packw(irels),
                "metau": build_meta(ub_band, ub, uslot, c_u, GB_U, RI),
                "metai": build_meta(ib_band, ib, islot, c_i, GB_I, RU),
                "iota": iota,
            }
        )
        asm.append((ub_band, ib_band))
    return in_maps, asm


def _build_program(ub_groups, ib_groups, c_u, c_i):
    from contextlib import ExitStack

    nc = bass.Bass()
    xiT = nc.declare_dram_parameter("xiT", [TI, P, KT, ROWTILE], BF16, isOutput=False)
    xuT = nc.declare_dram_parameter("xuT", [TU, P, KT, ROWTILE], BF16, isOutput=False)
    wv_d = nc.declare_dram_parameter("wv", [P, KT * 2 * SUB], BF16, isOutput=False)
    wu_d = nc.declare_dram_parameter("wu", [P, KT * 2 * SUB], BF16, isOutput=False)
    metau_d = nc.declare_dram_parameter("metau", [P, 3 * ub_groups * c_u], I32, isOutput=False)
    metai_d = nc.declare_dram_parameter("metai", [P, 3 * ib_groups * c_i], I32, isOutput=False)
    iota_d = nc.declare_dram_parameter("iota", [P, P], BF16, isOutput=False)
    ou = nc.declare_dram_parameter("ou", [ub_groups * P, SUB], F32, isOutput=True)
    oi = nc.declare_dram_parameter("oi", [ib_groups * P, SUB], F32, isOutput=True)
    vt = nc.dram_tensor("vt", [2 * RI, SUB], BF16)
    ut = nc.dram_tensor("ut", [2 * RU, SUB], BF16)

    ctx = ExitStack()
    sb = lambda name, shape, dt: ctx.enter_context(nc.sbuf_tensor(name, shape, dt))
    pt = lambda name, shape: ctx.enter_context(nc.psum_tensor(name, shape, F32))

    xt_s = sb("xt_s", [P, 2 * KT * ROWTILE], BF16)
    wv_s = sb("wv_s", [P, KT * 2 * SUB], BF16)
    wu_s = sb("wu_s", [P, KT * 2 * SUB], BF16)
    iota_s = sb("iota_s", [P, P], BF16)
    metau_s = sb("metau_s", [P, 3 * ub_groups * c_u], I32)
    metai_s = sb("metai_s", [P, 3 * ib_groups * c_i], I32)
    sbc_s = sb("sbc_s", [P, 4 * 2 * SUB], BF16)
    gt_s = sb("gt_s", [P, 8 * SUB], BF16)
    p_s = sb("p_s", [P, 8 * P], BF16)
    ob_s = sb("ob_s", [P, 8 * SUB], F32)

    ps1 = [pt(f"ps1_{i}", [P, 2 * SUB]) for i in range(4)]
    pse = [pt(f"pse_{i}", [P, SUB]) for i in range(4)]

    n_sub = (TI + TU) * 8
    n_sub_i = TI * 8
    u_batches = ub_groups // GB_U
    i_batches = ib_groups // GB_I
    n_groups = ub_groups + ib_groups
    n_chunks_u = ub_groups * c_u
    n_chunks = n_chunks_u + ib_groups * c_i

    def group_of_chunk(ch):
        if ch < n_chunks_u:
            return ch // c_u
        return ub_groups + (ch - n_chunks_u) // c_i

    with (
        nc.Block() as block,
        nc.semaphore("s_w") as s_w,
        nc.semaphore("s_xt") as s_xt,
        nc.semaphore("s_s1mm") as s_s1mm,
        nc.semaphore("s_cast") as s_cast,
        nc.semaphore("s_twr") as s_twr,
        nc.semaphore("s_meta") as s_meta,
        nc.semaphore("s_gath") as s_gath,
        nc.semaphore("s_p") as s_p,
        nc.semaphore("s_emm") as s_emm,
        nc.semaphore("s_relu") as s_relu,
        nc.semaphore("s_out") as s_out,
    ):
        # ---------------- SP: loads, table writes, output writes --------
        @block.sync
        def _(sp):
            sp.dma_start(out=wv_s[:, :], in_=wv_d[:, :]).then_inc(s_w, 16)
            sp.dma_start(out=wu_s[:, :], in_=wu_d[:, :]).then_inc(s_w, 16)

            total_tiles = TI + TU

            def tile_src(n):
                return xiT[n][:, :, :] if n < TI else xuT[n - TI][:, :, :]

            def load_tile(n):
                slot = n % 2
                if n >= 2:
                    sp.wait_ge(s_s1mm, 8 * (n - 1))
                sp.dma_start(
                    out=xt_s[:, slot * KT * ROWTILE : (slot + 1) * KT * ROWTILE],
                    in_=tile_src(n),
                ).then_inc(s_xt, 16)

            load_tile(0)
            load_tile(1)
            sub_idx = 0
            for n in range(total_tiles):
                tbl = vt if n < TI else ut
                rbase = (n if n < TI else n - TI) * ROWTILE
                rows_tot = RI if n < TI else RU
                for r8 in range(8):
                    s = sub_idx
                    sp.wait_ge(s_cast, s + 1)
                    slot = s % 4
                    row0 = rbase + r8 * P
                    for ci in range(2):
                        sp.dma_start(
                            out=tbl[ci * rows_tot + row0 : ci * rows_tot + row0 + P, :],
                            in_=sbc_s[:, slot * 2 * SUB + ci * SUB : slot * 2 * SUB + (ci + 1) * SUB],
                        ).then_inc(s_twr, 16)
                    sub_idx += 1
                if n + 2 < total_tiles:
                    load_tile(n + 2)

            for g in range(n_groups):
                sp.wait_ge(s_relu, g + 1)
                slot = g % 8
                if g < ub_groups:
                    dst = ou[g * P : (g + 1) * P, :]
                else:
                    gg = g - ub_groups
                    dst = oi[gg * P : (gg + 1) * P, :]
                sp.dma_start(
                    out=dst, in_=ob_s[:, slot * SUB : (slot + 1) * SUB]
                ).then_inc(s_out, 16)
            sp.wait_ge(s_out, 16 * n_groups)

        # ---------------- POOL: meta loads + gathers (per chunk) --------
        @block.gpsimd
        def _(gp):
            gp.dma_start(out=metau_s[:, :], in_=metau_d[:, :]).then_inc(s_meta, 16)
            gp.dma_start(out=metai_s[:, :], in_=metai_d[:, :]).then_inc(s_meta, 16)
            gp.dma_start(out=iota_s[:, :], in_=iota_d[:, :]).then_inc(s_meta, 16)
            gp.wait_ge(s_meta, 48)

            gp.wait_ge(s_twr, 16 * 2 * n_sub)  # all tables complete
            for ch in range(n_chunks):
                if ch >= 8:
                    gp.wait_ge(s_emm, group_of_chunk(ch - 8) + 1)
                if ch < n_chunks_u:
                    off = metau_s[:, ch : ch + 1]
                    src = vt
                else:
                    off = metai_s[:, ch - n_chunks_u : ch - n_chunks_u + 1]
                    src = ut
                gp.indirect_dma_start(
                    out=gt_s[:, (ch % 8) * SUB : (ch % 8 + 1) * SUB],
                    out_offset=None,
                    in_=src[:, :],
                    in_offset=bass.IndirectOffsetOnAxis(ap=off, axis=0),
                ).then_inc(s_gath, 16)

        # ---------------- DVE: P builds ---------------------------------
        @block.vector
        def _(dv):
            dv.wait_ge(s_meta, 48)
            for ch in range(n_chunks):
                if ch >= 8:
                    dv.wait_ge(s_emm, group_of_chunk(ch - 8) + 1)
                slot = ch % 8
                if ch < n_chunks_u:
                    ms, base, cc = metau_s, n_chunks_u, ch
                else:
                    ms, base, cc = metai_s, n_chunks - n_chunks_u, ch - n_chunks_u
                lr_col = ms[:, base + cc : base + cc + 1].bitcast(F32)
                w_col = ms[:, 2 * base + cc : 2 * base + cc + 1].bitcast(F32)
                dv.tensor_scalar(
                    out=p_s[:, slot * P : (slot + 1) * P],
                    in0=iota_s[:, :],
                    scalar1=lr_col,
                    scalar2=w_col,
                    op0=mybir.AluOpType.is_equal,
                    op1=mybir.AluOpType.mult,
                ).then_inc(s_p, 1)

        # ---------------- PE: stage-1 + edge matmuls --------------------
        @block.tensor
        def _(pe):
            pe.wait_ge(s_w, 32)
            sub_idx = 0
            for n in range(TI + TU):
                pe.wait_ge(s_xt, 16 * (n + 1))
                slot2 = n % 2
                w_sb = wv_s if n < TI else wu_s
                for r8 in range(8):
                    s = sub_idx
                    if s >= 4:
                        pe.wait_ge(s_cast, s - 3)
                    psd = ps1[s % 4]
                    mm = None
                    for k in range(KT):
                        mm = pe.matmul(
                            out=psd[:, :],
                            lhsT=xt_s[:, slot2 * KT * ROWTILE + k * ROWTILE + r8 * P : slot2 * KT * ROWTILE + k * ROWTILE + (r8 + 1) * P],
                            rhs=w_sb[:, k * 2 * SUB : (k + 1) * 2 * SUB],
                            start=(k == 0),
                            stop=(k == KT - 1),
                        )
                    mm.then_inc(s_s1mm, 1)
                    sub_idx += 1

            ch = 0
            for g in range(n_groups):
                user = g < ub_groups
                c_k = c_u if user else c_i
                if g >= 4:
                    pe.wait_ge(s_relu, g - 3)
                pe.wait_ge(s_p, ch + c_k)
                pe.wait_ge(s_gath, 16 * min(ch + 8, n_chunks))
                psd = pse[g % 4]
                mm = None
                for k in range(c_k):
                    mm = pe.matmul(
                        out=psd[:, :],
                        lhsT=p_s[:, ((ch + k) % 8) * P : ((ch + k) % 8 + 1) * P],
                        rhs=gt_s[:, ((ch + k) % 8) * SUB : ((ch + k) % 8 + 1) * SUB],
                        start=(k == 0),
                        stop=(k == c_k - 1),
                    )
                mm.then_inc(s_emm, 1)
                ch += c_k

        # ---------------- ACT: stage-1 casts + relus --------------------
        @block.scalar
        def _(ac):
            for s in range(n_sub):
                ac.wait_ge(s_s1mm, s + 1)
                if s >= 4:
                    ac.wait_ge(s_twr, 16 * 2 * (s - 3))
                slot = s % 4
                ac.copy(
                    out=sbc_s[:, slot * 2 * SUB : (slot + 1) * 2 * SUB],
                    in_=ps1[s % 4][:, :],
                ).then_inc(s_cast, 1)
            for g in range(n_groups):
                ac.wait_ge(s_emm, g + 1)
                if g >= 8:
                    ac.wait_ge(s_out, 16 * (g - 7))
                ac.activation(
                    out=ob_s[:, (g % 8) * SUB : (g % 8 + 1) * SUB],
                    in_=pse[g % 4][:, :],
                    func=mybir.ActivationFunctionType.Relu,
                ).then_inc(s_relu, 1)

    ctx.close()
    return nc


def kernel(**inputs):
    u_rows = np.asarray(inputs["user_sup_rows"])
    u_cols = np.asarray(inputs["user_sup_cols"])
    u_vals = np.asarray(inputs["user_sup_vals"])
    i_rows = np.asarray(inputs["item_sup_rows"])
    i_cols = np.asarray(inputs["item_sup_cols"])
    i_vals = np.asarray(inputs["item_sup_vals"])

    def cmax(rows_all, n_groups):
        m = 0
        for r in range(S):
            cnt = np.bincount(np.asarray(rows_all[r]) // P, minlength=n_groups)
            m = max(m, int(cnt.max()))
        return (m + P - 1) // P

    c_u = max(2, cmax(u_rows, GU))
    c_i = max(2, cmax(i_rows, GI))
    assert c_u <= 8 and c_i <= 8

    ub_groups = -(-S * GU // (NCORES * GB_U)) * GB_U
    ib_groups = -(-S * GI // (NCORES * GB_I)) * GB_I

    in_maps, asm = _host_prep(
        inputs["user_inputs"], inputs["item_inputs"], inputs["weight"],
        u_rows, u_cols, u_vals, i_rows, i_cols, i_vals,
        ub_groups, ib_groups, c_u, c_i,
    )
    nc = _build_program(ub_groups, ib_groups, c_u, c_i)
    _t0 = time.time()
    res = run_bass_kernel_spmd(nc, in_maps, core_ids=list(range(NCORES)))
    global LAST_RESULTS, EXEC_WALL_S
    EXEC_WALL_S = time.time() - _t0
    LAST_RESULTS = res

    out_u = np.zeros((S, GU * P, SUB), np.float32)
    out_i = np.zeros((S, GI * P, SUB), np.float32)
    for c in range(NCORES):
        ub_band, ib_band = asm[c]
        rou = np.asarray(res.results[c]["ou"]).reshape(ub_groups, P, SUB)
        roi = np.asarray(res.results[c]["oi"]).reshape(ib_groups, P, SUB)
        for b_pos, (r, g) in enumerate(ub_band):
            if r is not None:
                out_u[r, g * P : (g + 1) * P] = rou[b_pos]
        for b_pos, (r, g) in enumerate(ib_band):
            if r is not None:
                out_i[r, g * P : (g + 1) * P] = roi[b_pos]
    import os
    if os.environ.get("KDUMP"):
        np.savez("/tmp/kdump.npz",
                 **{f"ou{c}": np.asarray(res.results[c]["ou"]) for c in range(NCORES)},
                 **{f"oi{c}": np.asarray(res.results[c]["oi"]) for c in range(NCORES)})
    user_h = out_u.transpose(1, 0, 2).reshape(GU * P, O)[:NU]
    item_h = out_i.transpose(1, 0, 2).reshape(GI * P, O)[:NI]
    return user_h, item_h


# revision 12
# speedup vs baseline: 3.1354x; 3.1354x over previous
"""CatGCN encoder kernel for Trainium2 (8 NeuronCores, SPMD).

  user_h = relu(concat_s A_s @ (item_inputs @ W_s)),  item_h symmetric.

Per core: build the 1-2 bf16 table chunks (V_s = item_inputs@W_s resp.
U_s = user_inputs@W_s) its edge workload gathers from (stage 1, X^T
stationary matmuls over host-pretransposed bf16 inputs), then process
destination groups of 128 rows: indirect-DMA gather of source rows
(bf16, 200B, OOB-skip padding), a one-hot*weight selection matrix P per
128-edge chunk built on DVE (tensor_scalar is_equal*mult vs iota), PE
matmul P^T @ gathered accumulating [128,100] in PSUM, ACT relu, SP
writes the output tile.  Work is split into equal contiguous bands of
(relation, dest-group) atoms -> identical SPMD program on all 8 cores.
Raw Block style; all cross-engine waits are standalone wait_ge (this
toolchain allows only one sync wait per instruction).

Measured performance model (this container, trn2 via axon):
  - Each indirect (dynamic) DMA costs ~43us flat on the GPSIMD/SWDGE path
    regardless of payload/offset layout/queue count (bounds_check adds
    +40%, so padding uses idx 0 + w 0 instead).  The ~2706 per-chunk
    gather calls per core therefore dominate (~115 ms); all engine work
    (PE matmuls, DVE P-builds, ACT relus, stage-1, output DMA) hides
    underneath.  Measured minimum possible call count for this primitive
    on this data is 2703 (data-tight schedule saves 0.1%).
  - dma_gather (bulk CounterMachine path, int16 idx) would cut this to
    sub-ms but its custom Q7 ucode is absent here (device-unrecoverable
    crash).  Multi-index indirect_dma_start has different HW semantics
    than the simulator: one offset per dest partition, remaining free
    bytes stream contiguously from that row.
  - Gathers must trail table writes and lead PE by the full 8-slot ring:
    DMA completion counts skew across the 16 SDMA engines, and a tight
    wait races (corrupted ~15% of groups before the slack was added).
"""

import os
import time

import numpy as np
import ml_dtypes
import sys

sys.path.insert(0, "/opt/trn_rl_repo")

# NTFF profiling hook is unavailable in this container; a stray BASS_TRACE
# would crash run_bass_kernel_spmd under axon.
os.environ["BASS_NEVER_TRACE"] = "1"

EXEC_WALL_S = None

import concourse.bass as bass
from concourse import mybir
from concourse.bass_utils import run_bass_kernel_spmd

BF16 = mybir.dt.bfloat16
F32 = mybir.dt.float32
I32 = mybir.dt.int32

NU, NI, D, O, S, E = 100000, 50000, 512, 500, 5, 200000
SUB = O // S          # 100
NCORES = 8
P = 128
KT = D // P           # 4 k-tiles
ROWTILE = 1024
TI = 49               # item row tiles (49*1024 = 50176 >= NI)
TU = 98               # user row tiles (98*1024 = 100352 >= NU)
RI = TI * ROWTILE
RU = TU * ROWTILE
GI = RI // P          # item dest groups per relation (392)
GU = RU // P          # user dest groups per relation (784)
PADIDX = 1 << 28

GB_U = 4              # user groups per gather batch
GB_I = 2              # item groups per gather batch

LAST_RESULTS = None


def _host_prep(user_inputs, item_inputs, weight,
               u_rows, u_cols, u_vals, i_rows, i_cols, i_vals,
               ub_groups, ib_groups, c_u, c_i):
    w16 = np.asarray(weight).astype(ml_dtypes.bfloat16)

    def pretile(x, nt):
        rows = nt * ROWTILE
        xt = np.zeros((D, rows), dtype=ml_dtypes.bfloat16)
        xt[:, : x.shape[0]] = np.asarray(x).astype(ml_dtypes.bfloat16).T
        return np.ascontiguousarray(
            xt.reshape(KT, P, nt, ROWTILE).transpose(2, 1, 0, 3)
        )  # [nt][p][k][c]

    xiT = pretile(item_inputs, TI)
    xuT = pretile(user_inputs, TU)
    iota = np.broadcast_to(np.arange(P, dtype=ml_dtypes.bfloat16), (P, P)).copy()

    u_atoms = [(r, g) for r in range(S) for g in range(GU)]
    i_atoms = [(r, g) for r in range(S) for g in range(GI)]
    while len(u_atoms) < NCORES * ub_groups:
        u_atoms.append((None, None))
    while len(i_atoms) < NCORES * ib_groups:
        i_atoms.append((None, None))

    def bucket(rows_all, cols_all, vals_all, n_groups):
        out = []
        for r in range(S):
            rows = np.asarray(rows_all[r])
            order = np.argsort(rows, kind="stable")
            rs = rows[order]
            cs = np.asarray(cols_all[r])[order]
            vs = np.asarray(vals_all[r])[order]
            g = rs // P
            starts = np.searchsorted(g, np.arange(n_groups))
            ends = np.searchsorted(g, np.arange(n_groups) + 1)
            out.append((rs, cs, vs, starts, ends))
        return out

    ub = bucket(u_rows, u_cols, u_vals, GU)
    ib = bucket(i_rows, i_cols, i_vals, GI)

    in_maps, asm = [], []
    for c in range(NCORES):
        ub_band = u_atoms[c * ub_groups : (c + 1) * ub_groups]
        ib_band = i_atoms[c * ib_groups : (c + 1) * ib_groups]
        urels = sorted({r for r, _ in ub_band if r is not None})
        irels = sorted({r for r, _ in ib_band if r is not None})
        assert len(urels) <= 2 and len(irels) <= 2, (urels, irels)
        while len(urels) < 2:
            urels.append(urels[0] if urels else 0)
        while len(irels) < 2:
            irels.append(irels[0] if irels else 0)
        uslot = {urels[0]: 0, urels[1]: 1}
        islot = {irels[0]: 0, irels[1]: 1}

        def packw(rels):
            cols = np.concatenate(
                [w16[:, r * SUB : (r + 1) * SUB] for r in rels], axis=1
            )  # [512, 200]
            return np.ascontiguousarray(
                cols.reshape(KT, P, 2 * SUB).transpose(1, 0, 2).reshape(P, KT * 2 * SUB)
            )

        def build_meta(band, buckets, slot_of, c_k, gb, table_rows):
            ncols = len(band) * c_k
            idx = np.zeros((P, ncols), dtype=np.int64)
            lr = np.zeros((P, ncols), dtype=np.float32)
            wt = np.zeros((P, ncols), dtype=np.float32)
            for b_pos, (r, g) in enumerate(band):
                if r is None:
                    continue
                rs, cs, vs, starts, ends = buckets[r]
                s0, e0 = int(starts[g]), int(ends[g])
                n = e0 - s0
                assert n <= c_k * P, f"group overflow {n} > {c_k * P}"
                j = np.arange(n)
                pp = j % P
                col = b_pos * c_k + j // P
                idx[pp, col] = slot_of[r] * table_rows + cs[s0:e0]
                lr[pp, col] = (rs[s0:e0] - g * P).astype(np.float32)
                wt[pp, col] = vs[s0:e0]
            return np.ascontiguousarray(
                np.concatenate(
                    [idx.astype(np.int32), lr.view(np.int32), wt.view(np.int32)],
                    axis=1,
                )
            )

        in_maps.append(
            {
                "xiT": xiT, "xuT": xuT,
                "wv": packw(urels), "wu": packw(irels),
                "metau": build_meta(ub_band, ub, uslot, c_u, GB_U, RI),
                "metai": build_meta(ib_band, ib, islot, c_i, GB_I, RU),
                "iota": iota,
            }
        )
        asm.append((ub_band, ib_band))
    return in_maps, asm


def _build_program(ub_groups, ib_groups, c_u, c_i):
    from contextlib import ExitStack

    nc = bass.Bass()
    xiT = nc.declare_dram_parameter("xiT", [TI, P, KT, ROWTILE], BF16, isOutput=False)
    xuT = nc.declare_dram_parameter("xuT", [TU, P, KT, ROWTILE], BF16, isOutput=False)
    wv_d = nc.declare_dram_parameter("wv", [P, KT * 2 * SUB], BF16, isOutput=False)
    wu_d = nc.declare_dram_parameter("wu", [P, KT * 2 * SUB], BF16, isOutput=False)
    metau_d = nc.declare_dram_parameter("metau", [P, 3 * ub_groups * c_u], I32, isOutput=False)
    metai_d = nc.declare_dram_parameter("metai", [P, 3 * ib_groups * c_i], I32, isOutput=False)
    iota_d = nc.declare_dram_parameter("iota", [P, P], BF16, isOutput=False)
    ou = nc.declare_dram_parameter("ou", [ub_groups * P, SUB], F32, isOutput=True)
    oi = nc.declare_dram_parameter("oi", [ib_groups * P, SUB], F32, isOutput=True)
    vt = nc.dram_tensor("vt", [2 * RI, SUB], BF16)
    ut = nc.dram_tensor("ut", [2 * RU, SUB], BF16)

    ctx = ExitStack()
    sb = lambda name, shape, dt: ctx.enter_context(nc.sbuf_tensor(name, shape, dt))
    pt = lambda name, shape: ctx.enter_context(nc.psum_tensor(name, shape, F32))

    xt_s = sb("xt_s", [P, 2 * KT * ROWTILE], BF16)
    wv_s = sb("wv_s", [P, KT * 2 * SUB], BF16)
    wu_s = sb("wu_s", [P, KT * 2 * SUB], BF16)
    iota_s = sb("iota_s", [P, P], BF16)
    metau_s = sb("metau_s", [P, 3 * ub_groups * c_u], I32)
    metai_s = sb("metai_s", [P, 3 * ib_groups * c_i], I32)
    sbc_s = sb("sbc_s", [P, 4 * 2 * SUB], BF16)
    gt_s = sb("gt_s", [P, 8 * SUB], BF16)
    p_s = sb("p_s", [P, 8 * P], BF16)
    ob_s = sb("ob_s", [P, 8 * SUB], F32)

    ps1 = [pt(f"ps1_{i}", [P, 2 * SUB]) for i in range(4)]
    pse = [pt(f"pse_{i}", [P, SUB]) for i in range(4)]

    n_sub = (TI + TU) * 8
    n_sub_i = TI * 8
    u_batches = ub_groups // GB_U
    i_batches = ib_groups // GB_I
    n_groups = ub_groups + ib_groups
    n_chunks_u = ub_groups * c_u
    n_chunks = n_chunks_u + ib_groups * c_i

    def group_of_chunk(ch):
        if ch < n_chunks_u:
            return ch // c_u
        return ub_groups + (ch - n_chunks_u) // c_i

    with (
        nc.Block() as block,
        nc.semaphore("s_w") as s_w,
        nc.semaphore("s_xt") as s_xt,
        nc.semaphore("s_s1mm") as s_s1mm,
        nc.semaphore("s_cast") as s_cast,
        nc.semaphore("s_twr") as s_twr,
        nc.semaphore("s_meta") as s_meta,
        nc.semaphore("s_gath") as s_gath,
        nc.semaphore("s_p") as s_p,
        nc.semaphore("s_emm") as s_emm,
        nc.semaphore("s_relu") as s_relu,
        nc.semaphore("s_out") as s_out,
    ):
        # ---------------- SP: loads, table writes, output writes --------
        @block.sync
        def _(sp):
            sp.dma_start(out=wv_s[:, :], in_=wv_d[:, :]).then_inc(s_w, 16)
            sp.dma_start(out=wu_s[:, :], in_=wu_d[:, :]).then_inc(s_w, 16)

            total_tiles = TI + TU

            def tile_src(n):
                return xiT[n][:, :, :] if n < TI else xuT[n - TI][:, :, :]

            def load_tile(n):
                slot = n % 2
                if n >= 2:
                    sp.wait_ge(s_s1mm, 8 * (n - 1))
                sp.dma_start(
                    out=xt_s[:, slot * KT * ROWTILE : (slot + 1) * KT * ROWTILE],
                    in_=tile_src(n),
                ).then_inc(s_xt, 16)

            load_tile(0)
            load_tile(1)
            sub_idx = 0
            for n in range(total_tiles):
                tbl = vt if n < TI else ut
                rbase = (n if n < TI else n - TI) * ROWTILE
                rows_tot = RI if n < TI else RU
                for r8 in range(8):
                    s = sub_idx
                    sp.wait_ge(s_cast, s + 1)
                    slot = s % 4
                    row0 = rbase + r8 * P
                    for ci in range(2):
                        sp.dma_start(
                            out=tbl[ci * rows_tot + row0 : ci * rows_tot + row0 + P, :],
                            in_=sbc_s[:, slot * 2 * SUB + ci * SUB : slot * 2 * SUB + (ci + 1) * SUB],
                        ).then_inc(s_twr, 16)
                    sub_idx += 1
                if n + 2 < total_tiles:
                    load_tile(n + 2)

            for g in range(n_groups):
                sp.wait_ge(s_relu, g + 1)
                slot = g % 8
                if g < ub_groups:
                    dst = ou[g * P : (g + 1) * P, :]
                else:
                    gg = g - ub_groups
                    dst = oi[gg * P : (gg + 1) * P, :]
                sp.dma_start(
                    out=dst, in_=ob_s[:, slot * SUB : (slot + 1) * SUB]
                ).then_inc(s_out, 16)
            sp.wait_ge(s_out, 16 * n_groups)

        # ---------------- POOL: meta loads + gathers (per chunk) --------
        @block.gpsimd
        def _(gp):
            gp.dma_start(out=metau_s[:, :], in_=metau_d[:, :]).then_inc(s_meta, 16)
            gp.dma_start(out=metai_s[:, :], in_=metai_d[:, :]).then_inc(s_meta, 16)
            gp.dma_start(out=iota_s[:, :], in_=iota_d[:, :]).then_inc(s_meta, 16)
            gp.wait_ge(s_meta, 48)

            gp.wait_ge(s_twr, 16 * 2 * n_sub)  # all tables complete
            for ch in range(n_chunks):
                if ch >= 8:
                    gp.wait_ge(s_emm, group_of_chunk(ch - 8) + 1)
                if ch < n_chunks_u:
                    off = metau_s[:, ch : ch + 1]
                    src = vt
                else:
                    off = metai_s[:, ch - n_chunks_u : ch - n_chunks_u + 1]
                    src = ut
                gp.indirect_dma_start(
                    out=gt_s[:, (ch % 8) * SUB : (ch % 8 + 1) * SUB],
                    out_offset=None,
                    in_=src[:, :],
                    in_offset=bass.IndirectOffsetOnAxis(ap=off, axis=0),
                ).then_inc(s_gath, 16)

        # ---------------- DVE: P builds ---------------------------------
        @block.vector
        def _(dv):
            dv.wait_ge(s_meta, 48)
            for ch in range(n_chunks):
                if ch >= 8:
                    dv.wait_ge(s_emm, group_of_chunk(ch - 8) + 1)
                slot = ch % 8
                if ch < n_chunks_u:
                    ms, base, cc = metau_s, n_chunks_u, ch
                else:
                    ms, base, cc = metai_s, n_chunks - n_chunks_u, ch - n_chunks_u
                lr_col = ms[:, base + cc : base + cc + 1].bitcast(F32)
                w_col = ms[:, 2 * base + cc : 2 * base + cc + 1].bitcast(F32)
                dv.tensor_scalar(
                    out=p_s[:, slot * P : (slot + 1) * P],
                    in0=iota_s[:, :],
                    scalar1=lr_col,
                    scalar2=w_col,
                    op0=mybir.AluOpType.is_equal,
                    op1=mybir.AluOpType.mult,
                ).then_inc(s_p, 1)

        # ---------------- PE: stage-1 + edge matmuls --------------------
        @block.tensor
        def _(pe):
            pe.wait_ge(s_w, 32)
            sub_idx = 0
            for n in range(TI + TU):
                pe.wait_ge(s_xt, 16 * (n + 1))
                slot2 = n % 2
                w_sb = wv_s if n < TI else wu_s
                for r8 in range(8):
                    s = sub_idx
                    if s >= 4:
                        pe.wait_ge(s_cast, s - 3)
                    psd = ps1[s % 4]
                    mm = None
                    for k in range(KT):
                        mm = pe.matmul(
                            out=psd[:, :],
                            lhsT=xt_s[:, slot2 * KT * ROWTILE + k * ROWTILE + r8 * P : slot2 * KT * ROWTILE + k * ROWTILE + (r8 + 1) * P],
                            rhs=w_sb[:, k * 2 * SUB : (k + 1) * 2 * SUB],
                            start=(k == 0),
                            stop=(k == KT - 1),
                        )
                    mm.then_inc(s_s1mm, 1)
                    sub_idx += 1

            ch = 0
            for g in range(n_groups):
                user = g < ub_groups
                c_k = c_u if user else c_i
                if g >= 4:
                    pe.wait_ge(s_relu, g - 3)
                pe.wait_ge(s_p, ch + c_k)
                pe.wait_ge(s_gath, 16 * min(ch + 8, n_chunks))
                psd = pse[g % 4]
                mm = None
                for k in range(c_k):
                    mm = pe.matmul(
                        out=psd[:, :],
                        lhsT=p_s[:, ((ch + k) % 8) * P : ((ch + k) % 8 + 1) * P],
                        rhs=gt_s[:, ((ch + k) % 8) * SUB : ((ch + k) % 8 + 1) * SUB],
                        start=(k == 0),
                        stop=(k == c_k - 1),
                    )
                mm.then_inc(s_emm, 1)
                ch += c_k

        # ---------------- ACT: stage-1 casts + relus --------------------
        @block.scalar
        def _(ac):
            for s in range(n_sub):
                ac.wait_ge(s_s1mm, s + 1)
                if s >= 4:
                    ac.wait_ge(s_twr, 16 * 2 * (s - 3))
                slot = s % 4
                ac.copy(
                    out=sbc_s[:, slot * 2 * SUB : (slot + 1) * 2 * SUB],
                    in_=ps1[s % 4][:, :],
                ).then_inc(s_cast, 1)
            for g in range(n_groups):
                ac.wait_ge(s_emm, g + 1)
                if g >= 8:
                    ac.wait_ge(s_out, 16 * (g - 7))
                ac.activation(
                    out=ob_s[:, (g % 8) * SUB : (g % 8 + 1) * SUB],
                    in_=pse[g % 4][:, :],
                    func=mybir.ActivationFunctionType.Relu,
                ).then_inc(s_relu, 1)

    ctx.close()
    return nc


def kernel(**inputs):
    u_rows = np.asarray(inputs["user_sup_rows"])
    u_cols = np.asarray(inputs["user_sup_cols"])
    u_vals = np.asarray(inputs["user_sup_vals"])
    i_rows = np.asarray(inputs["item_sup_rows"])
    i_cols = np.asarray(inputs["item_sup_cols"])
    i_vals = np.asarray(inputs["item_sup_vals"])

    def cmax(rows_all, n_groups):
        m = 0
        for r in range(S):
            cnt = np.bincount(np.asarray(rows_all[r]) // P, minlength=n_groups)
            m = max(m, int(cnt.max()))
        return (m + P - 1) // P

    c_u = max(2, cmax(u_rows, GU))
    c_i = max(2, cmax(i_rows, GI))
    assert c_u <= 8 and c_i <= 8

    ub_groups = -(-S * GU // (NCORES * GB_U)) * GB_U
    ib_groups = -(-S * GI // (NCORES * GB_I)) * GB_I

    in_maps, asm = _host_prep(
        inputs["user_inputs"], inputs["item_inputs"], inputs["weight"],
        u_rows, u_cols, u_vals, i_rows, i_cols, i_vals,
        ub_groups, ib_groups, c_u, c_i,
    )
    nc = _build_program(ub_groups, ib_groups, c_u, c_i)
    _t0 = time.time()
    res = run_bass_kernel_spmd(nc, in_maps, core_ids=list(range(NCORES)))
    global LAST_RESULTS, EXEC_WALL_S
    EXEC_WALL_S = time.time() - _t0
    LAST_RESULTS = res

    out_u = np.zeros((S, GU * P, SUB), np.float32)
    out_i = np.zeros((S, GI * P, SUB), np.float32)
    for c in range(NCORES):
        ub_band, ib_band = asm[c]
        rou = np.asarray(res.results[c]["ou"]).reshape(ub_groups, P, SUB)
        roi = np.asarray(res.results[c]["oi"]).reshape(ib_groups, P, SUB)
        for b_pos, (r, g) in enumerate(ub_band):
            if r is not None:
                out_u[r, g * P : (g + 1) * P] = rou[b_pos]
        for b_pos, (r, g) in enumerate(ib_band):
            if r is not None:
                out_i[r, g * P : (g + 1) * P] = roi[b_pos]
    import os
    if os.environ.get("KDUMP"):
        np.savez("/tmp/kdump.npz",
                 **{f"ou{c}": np.asarray(res.results[c]["ou"]) for c in range(NCORES)},
                 **{f"oi{c}": np.asarray(res.results[c]["oi"]) for c in range(NCORES)})
    user_h = out_u.transpose(1, 0, 2).reshape(GU * P, O)[:NU]
    item_h = out_i.transpose(1, 0, 2).reshape(GI * P, O)[:NI]
    return user_h, item_h
